# revision 16
# baseline (speedup 1.0000x reference)
"""Trainium2 Bass kernel for nn_CoreDecoderStatefull — v3.

Single-step stateful decoder: dense -> 5x [GRU -> GLU -> concat -> conv1d(k=2)
-> concat] -> out projection.  batch=1; latency-bound serial chain.

v3 changes vs v2 (39.7us):
  * HW-measured cost model (loop-difference microbenchmarks): ACT op ~380ns
    dependent / ~254 independent; DVE tensor_scalar ~170ns but
    scalar_tensor_tensor ~328ns; LDW+MM pair 182ns @96-col stationary but
    70.7ns @128-col (FWL kicks in at NumWeights==128); DMA ~0.7-1.0us fixed
    per transfer, serialized.
  * All 96/80-col weight blocks padded to 128 cols -> FWL halves LDWEIGHTS
    and pairs pipeline 2.6x better.  (conv 32-col blocks stay unpadded.)
  * DMA: 10 transfers -> 5 need-ordered waves (head+stage1 / stile /
    stage2 / stages3-4 / stage5+out), each ~0.3-1.1MB.
  * GRU blend avoids scalar_tensor_tensor: hn = (c*onemu)+uhnh via ONE
    tensor_scalar with two per-partition AP scalars; uhnh=(u*h+nh) and
    onemu=(1-u) precomputed on DVE concurrently with the c tanh.
  * Psum split Prz/Pn per stage: boundary emits r,z cv-row matmuls first
    (stop) so the rz sigmoid starts before the n-gate matmul lands.
  * conv g-tap folded: pre-scale the [96,32] cw1_g block by clamped hn on
    DVE (off-chain, concurrent with GLU matmul) and matmul against sg
    directly -- the g=hn*sg multiply leaves the critical path.
  * x0 tanh writes bf16 XH col directly (no extra DVE move).
"""

import numpy as np
from contextlib import ExitStack

GD = [96, 224, 352, 480, 608]   # GRU input dims per stage
CD = [192, 320, 448, 576, 704]  # conv input dims per stage
N_CORES = 1
N_WAVES = 5


def _bf16(a):
    a = np.ascontiguousarray(np.asarray(a, np.float32))
    u = a.view(np.uint32)
    r = ((u + 0x7FFF + ((u >> 16) & 1)) & 0xFFFF0000).astype(np.uint32)
    return r.view(np.float32)


# ---------------------------------------------------------------------------
# x-vector index mapping: chunk c row r -> index in the reference concat x
# chunk c: rows 0:96 = x0 (c=0) or g_c, rows 96:128 = cv_{c+1}
# ---------------------------------------------------------------------------
def _refidx(c: int, r: int) -> int:
    if r < 96:
        return r if c == 0 else GD[c - 1] + r  # x0 / g_c
    return CD[c] + (r - 96)                    # cv_{c+1}


def _gi_chunks(s):
    # (chunk, rows): full chunks then the g_{s-1}-only last chunk
    return [(c, 128) for c in range(s - 1)] + [(s - 1, 96)]


def _cvx_chunks(s):
    # conv_s x-input chunks; chunk s (rows 0:96, g_s) is the runtime-scaled
    # g-tap; chunks 0..s-2 full, chunk s-1 g-only
    return [(c, 128) for c in range(s - 1)] + [(s - 1, 96), (s, 96)]


_OUT_CHUNKS = [(c, 128) for c in range(5)] + [(5, 97)]

# stage -> wave for that stage's chain weights
_STAGE_WAVE = {1: 0, 2: 1, 3: 2, 4: 3, 5: 4}


# ---------------------------------------------------------------------------
# static layout: 4 bf16 wave slabs + small f32 state tile
# ---------------------------------------------------------------------------
def _layout():
    wt = {}  # name -> (wave, col, rows, ncols)
    wave_cols = [0] * N_WAVES

    def put(name, wave, rows, ncols):
        wt[name] = (wave, wave_cols[wave], rows, ncols)
        wave_cols[wave] += ncols

    # wave 0: everything stage-1-critical; later stages' inputs ride the
    # same wave as that stage's weights
    put("zxh", 0, 81, 1)
    put("dense", 0, 81, 128)
    for s in range(1, 6):
        put(f"hxh{s}", _STAGE_WAVE[s], 97, 1)
    for s in range(1, 6):
        cd = CD[s - 1]
        nch = (cd + 127) // 128
        for j in range(nch):
            rows = min(128, cd - 128 * j)
            if j == nch - 1:
                rows += 1  # aug row = 1.0 (for cvc bias)
            put(f"cxh{s}_{j}", _STAGE_WAVE[s], rows, 1)

    for s in range(1, 6):
        w = _STAGE_WAVE[s]
        for j in range(3):
            put(f"gh{s}_{j}", w, 97, 128)
        for (c, rows) in _gi_chunks(s):
            for j in range(3):
                put(f"gi{s}_{c}_{j}", w, rows, 128)
        put(f"glu{s}", w, 96, 128)
        for (c, rows) in _cvx_chunks(s):
            put(f"cvx{s}_{c}", w, rows, 128 if c == s else 32)
        cd = CD[s - 1]
        nch = (cd + 127) // 128
        for j in range(nch):
            rows = min(128, cd - 128 * j)
            if j == nch - 1:
                rows += 1  # aug bias row
            put(f"cvc{s}_{j}", w, rows, 128 if j == 0 else 32)

    for (c, rows) in _OUT_CHUNKS:
        put(f"out{c}", 4, rows, 128)

    # stile: fp32 state columns (nh, hc, binf per stage)
    st = {}
    scol = 0
    for s in range(1, 6):
        for nm in (f"nh{s}", f"hc{s}", f"binf{s}"):
            st[nm] = scol
            scol += 1
    return wt, wave_cols, st, scol


_WT, _WAVE_COLS, _ST, _ST_COLS = _layout()


# ---------------------------------------------------------------------------
# host-side packing
# ---------------------------------------------------------------------------
def _noise_vectors():
    # NOTE: must follow the exact same jax.random path as the reference.
    import jax
    import jax.numpy as jnp

    vs = {}
    for i in range(16):
        n = 96 if (i == 0 or i % 3 != 0) else 32
        u = jax.random.uniform(
            jax.random.fold_in(jax.random.key(42), i), (1, n),
            dtype=jnp.float32
        )
        vs[i] = (np.asarray(u).reshape(-1) - 0.5) / np.float32(127.0)
    return vs


def _to_ml_bf16(a):
    import ml_dtypes

    return np.asarray(a, np.float32).astype(ml_dtypes.bfloat16)


def _pack(inp):
    f32 = np.float32
    nv = _noise_vectors()
    # nu_x: constant noise folded into the device x (x0, g_s, cv_s sites)
    nux = np.zeros(736, f32)
    nux[0:96] = nv[0]
    for s in range(1, 6):
        nux[GD[s - 1]:GD[s - 1] + 96] = nv[3 * s - 1]   # g_s
        nux[CD[s - 1]:CD[s - 1] + 32] = nv[3 * s]       # cv_s
    waves = [np.zeros((128, c), f32) for c in _WAVE_COLS]
    stile = np.zeros((128, _ST_COLS), f32)

    def wfill(name, block):
        wave, col, rows, ncols = _WT[name]
        assert block.shape[0] == rows, (name, block.shape, rows)
        assert block.shape[1] <= ncols, (name, block.shape, ncols)
        waves[wave][:rows, col:col + block.shape[1]] = _bf16(block)

    def sfill(name, vec):
        stile[:vec.shape[0], _ST[name]] = vec

    # dense: rows 0:80 = w.T, row 80 = bias
    blk = np.zeros((81, 96), f32)
    blk[:80] = inp["w_dense"].T
    blk[80] = inp["b_dense"]
    wfill("dense", blk)

    for s in range(1, 6):
        wi = inp[f"g{s}_wi"].astype(f32)
        wh = inp[f"g{s}_wh"].astype(f32)
        bi = inp[f"g{s}_bi"].astype(f32)
        bh = inp[f"g{s}_bh"].astype(f32)
        nux_s = nux[:GD[s - 1]]
        fold = wi @ nux_s  # (288,)
        # gh blocks: rows 0:96 wh.T, row 96 bias(+fold for r,z)
        for j in range(3):
            blk = np.zeros((97, 96), f32)
            blk[:96] = wh[96 * j:96 * (j + 1), :].T
            if j < 2:
                blk[96] = (bi + bh)[96 * j:96 * (j + 1)] + fold[96 * j:96 * (j + 1)]
            else:
                blk[96] = bh[192:288]
            wfill(f"gh{s}_{j}", blk)
        sfill(f"binf{s}", bi[192:288] + fold[192:288])
        sfill(f"nh{s}", nv[3 * s - 2])
        sfill(f"hc{s}", inp[f"h{s}"].reshape(-1).astype(f32))

        # gi chunks (row-permuted)
        for (c, rows) in _gi_chunks(s):
            ridx = [_refidx(c, r) for r in range(rows)]
            for j in range(3):
                wfill(f"gi{s}_{c}_{j}", wi[96 * j:96 * (j + 1), ridx].T)

        wfill(f"glu{s}", inp[f"glu{s}_w"].T.astype(f32))

        cw = inp[f"cv{s}_w"].astype(f32)
        cw0, cw1 = cw[:, :, 0], cw[:, :, 1]
        cd = CD[s - 1]
        # conv x-taps (chunk s = the g_s tap, runtime-scaled on device)
        for (c, rows) in _cvx_chunks(s):
            ridx = [_refidx(c, r) for r in range(rows)]
            wfill(f"cvx{s}_{c}", cw1[:, ridx].T)
        # conv c-taps; last chunk aug row = cb + cw1 @ nux fold
        nch = (cd + 127) // 128
        for j in range(nch):
            rows = min(128, cd - 128 * j)
            blk_w = cw0[:, 128 * j:128 * j + rows].T
            if j == nch - 1:
                blk = np.zeros((rows + 1, 32), f32)
                blk[:rows] = blk_w
                blk[rows] = inp[f"cv{s}_b"].astype(f32) + cw1 @ nux[:cd]
                wfill(f"cvc{s}_{j}", blk)
            else:
                wfill(f"cvc{s}_{j}", blk_w)

    w_out = inp["w_out"].astype(f32)
    for (c, rows) in _OUT_CHUNKS:
        if c < 5:
            ridx = [_refidx(c, r) for r in range(rows)]
            wfill(f"out{c}", w_out[:, ridx].T)
        else:
            blk = np.zeros((97, 80), f32)
            ridx = [GD[4] + r for r in range(96)]  # g5 dims
            blk[:96] = w_out[:, ridx].T
            blk[96] = inp["b_out"].astype(f32) + w_out @ nux
            wfill(f"out{c}", blk)

    # bf16 input columns (in wave 0)
    zv = np.zeros((81, 1), f32)
    zv[:80, 0] = inp["z"].reshape(-1)
    zv[80, 0] = 1.0
    wfill("zxh", zv)
    for s in range(1, 6):
        hv = np.zeros((97, 1), f32)
        hv[:96, 0] = inp[f"h{s}"].reshape(-1)
        hv[96, 0] = 1.0
        wfill(f"hxh{s}", hv)
        cv_in = inp[f"c{s}"].reshape(-1).astype(f32)
        cd = CD[s - 1]
        nch = (cd + 127) // 128
        for j in range(nch):
            rows = min(128, cd - 128 * j)
            if j == nch - 1:
                v = np.zeros((rows + 1, 1), f32)
                v[:rows, 0] = cv_in[128 * j:128 * j + rows]
                v[rows, 0] = 1.0
                wfill(f"cxh{s}_{j}", v)
            else:
                wfill(f"cxh{s}_{j}", cv_in[128 * j:128 * j + rows].reshape(-1, 1))

    m = {f"wave{i}": _to_ml_bf16(waves[i]) for i in range(N_WAVES)}
    m["stile"] = stile
    return m


# ---------------------------------------------------------------------------
# device program
# ---------------------------------------------------------------------------
def _build_nc(loop_iters=None, dma_only=False, compute_only=False,
              n_stages=5):
    from concourse import bacc, tile, mybir

    F32 = mybir.dt.float32
    BF16 = mybir.dt.bfloat16
    AF = mybir.ActivationFunctionType
    OP = mybir.AluOpType

    nc = bacc.Bacc("TRN2", target_bir_lowering=False, debug=False,
                   num_devices=N_CORES)
    wdram = {i: nc.dram_tensor(f"wave{i}", [128, _WAVE_COLS[i]], BF16,
                               kind="ExternalInput") for i in range(N_WAVES)}
    sdram = nc.dram_tensor("stile", [128, _ST_COLS], F32, kind="ExternalInput")
    ydram = nc.dram_tensor("y", [80, 1], F32, kind="ExternalOutput")

    with tile.TileContext(nc) as tc, ExitStack() as ctx:
        dbuf = 2 if loop_iters is not None else 1
        wpool = ctx.enter_context(tc.tile_pool(name="wpool", bufs=dbuf))
        spool = ctx.enter_context(tc.tile_pool(name="spool", bufs=dbuf))
        work = ctx.enter_context(tc.tile_pool(name="work", bufs=2))
        xpool = ctx.enter_context(tc.tile_pool(name="xpool", bufs=1))
        prz = ctx.enter_context(tc.tile_pool(name="prz", bufs=2, space="PSUM"))
        ppn = ctx.enter_context(tc.tile_pool(name="ppn", bufs=2, space="PSUM"))
        pgh = ctx.enter_context(tc.tile_pool(name="pgh", bufs=1, space="PSUM"))
        pcv = ctx.enter_context(tc.tile_pool(name="pcv", bufs=1, space="PSUM"))
        pq = ctx.enter_context(tc.tile_pool(name="pq", bufs=1, space="PSUM"))
        pout = ctx.enter_context(tc.tile_pool(name="pout", bufs=1, space="PSUM"))

        if loop_iters is not None:
            ctx.enter_context(tc.For_i(0, loop_iters, 1))

        # single warm sigmoid: pins the table set to sigmoid_and_others
        # (includes tanh) so exactly one ACT_TABLE_LOAD is emitted
        warm = work.tile([1, 1], F32, tag="warm", name="warm")
        nc.vector.memset(warm[:], 0.0)
        nc.scalar.activation(warm[:], warm[:], AF.Sigmoid)

        XH = xpool.tile([128, 6], BF16, tag="XH", name="XH")
        XHN = xpool.tile([96, 5], BF16, tag="XHN", name="XHN")
        SGB = xpool.tile([96, 5], BF16, tag="SGB", name="SGB")
        nc.vector.memset(XH[96:97, 5:6], 1.0)  # aug row for out bias

        stile = spool.tile([128, _ST_COLS], F32, tag="stile", name="stile")
        wt = {}
        for i in range(N_WAVES):
            wt[i] = wpool.tile([128, _WAVE_COLS[i]], BF16, tag=f"w{i}",
                               name=f"wt{i}")
        if not compute_only:
            nc.sync.dma_start(out=wt[0][:], in_=wdram[0][:])
            nc.sync.dma_start(out=stile[:], in_=sdram[:])
            for i in range(1, N_WAVES):
                nc.sync.dma_start(out=wt[i][:], in_=wdram[i][:])
        else:
            nc.vector.memset(stile[:, 0:1], 0.01)
            for i in range(N_WAVES):
                nc.vector.memset(wt[i][:, 0:1], 0.01)

        def W(name):
            wave, col, rows, ncols = _WT[name]
            return wt[wave][0:rows, col:col + ncols]

        def S(name):
            return stile[0:96, _ST[name]:_ST[name] + 1]

        if not dma_only:
            _emit_compute(nc, tc, work, xpool, prz, ppn, pgh, pcv, pq, pout,
                          W, S, XH, XHN, SGB, n_stages, mybir, ydram)

    nc.compile()
    return nc


def _emit_compute(nc, tc, work, xpool, prz, ppn, pgh, pcv, pq, pout,
                  W, S, XH, XHN, SGB, n_stages, mybir, ydram):
    F32 = mybir.dt.float32
    BF16 = mybir.dt.bfloat16
    AF = mybir.ActivationFunctionType
    OP = mybir.AluOpType

    Prz = {}
    Pn = {}
    ghn = {}
    phn = {}

    def emit_ghn_copy(t):
        ghn[t] = work.tile([96, 1], F32, tag="ghn", name=f"ghn{t}")
        nc.vector.tensor_copy(ghn[t][:], phn[t][0:96, :])

    def emit_gh(t):
        """gh r,z into Prz[t] (opens the group), gh n into its own psum tile
        (closed immediately; DVE copies it out right after)."""
        Prz[t] = prz.tile([128, 2], F32, tag="prz", name=f"Prz{t}")
        for j in range(2):
            nc.tensor.matmul(Prz[t][:, j:j + 1], W(f"gh{t}_{j}"),
                             W(f"hxh{t}"), start=(j == 0), stop=False)
        phn[t] = pgh.tile([128, 1], F32, tag="gh", name=f"Pghn{t}")
        nc.tensor.matmul(phn[t][:], W(f"gh{t}_2"), W(f"hxh{t}"),
                         start=True, stop=True)

    def gi_rz(t, c, rows, stop=False):
        for j in range(2):
            nc.tensor.matmul(Prz[t][:, j:j + 1], W(f"gi{t}_{c}_{j}"),
                             XH[0:rows, c:c + 1],
                             start=False, stop=(stop and j == 1))

    def gi_n(t, c, rows, start=False, stop=False):
        if start:
            Pn[t] = ppn.tile([128, 1], F32, tag="pn", name=f"Pn{t}")
        nc.tensor.matmul(Pn[t][:], W(f"gi{t}_{c}_2"), XH[0:rows, c:c + 1],
                         start=start, stop=stop)

    def cvx_mm(s, c, rows, stop=False):
        nc.tensor.matmul(pR[0:32, s - 1:s], W(f"cvx{s}_{c}"),
                         XH[0:rows, c:c + 1], start=False, stop=stop,
                         skip_group_check=(s > 1))

    # ---------------- head ----------------
    pd = pq.tile([128, 1], F32, tag="q", name="pdense")
    nc.tensor.matmul(pd[:], W("dense"), W("zxh"), start=True, stop=True)
    emit_gh(1)
    emit_ghn_copy(1)
    nc.scalar.activation(XH[0:96, 0:1], pd[0:96, :], AF.Tanh)  # x0 (bf16)

    gi_rz(1, 0, 96, stop=True)
    gi_n(1, 0, 96, start=True, stop=True)

    pR = pcv.tile([128, 5], F32, tag="cv", name="pR")
    for j in range((CD[0] + 127) // 128):
        nc.tensor.matmul(pR[0:128 if j == 0 else 32, 0:1], W(f"cvc1_{j}"),
                         W(f"cxh1_{j}"), start=(j == 0), stop=False)
    cvx_mm(1, 0, 96)

    O = pout.tile([128, 1], F32, tag="out", name="Oout")

    # ---------------- stage chain ----------------
    pn_started = {1: True}
    for s in range(1, n_stages + 1):
        # ---- DVE pre-chain (concurrent with rz/c ACT) ----
        ginb = work.tile([96, 1], F32, tag="ginb", name=f"ginb{s}")
        nc.vector.tensor_scalar(ginb[:], Pn[s][0:96, :], S(f"binf{s}"),
                                None, OP.add)
        rz = work.tile([96, 2], F32, tag="rz", name=f"rz{s}")
        nc.scalar.activation(rz[:], Prz[s][0:96, :], AF.Sigmoid)
        c_ = work.tile([96, 1], F32, tag="c_", name=f"c{s}_")
        nc.scalar.activation(c_[:], rz[:, 0:1], AF.Tanh,
                             bias=ginb[:], scale=ghn[s][:])
        # during c tanh: uhnh = u*h + nh ; onemu = 1 - u
        uhnh = work.tile([96, 1], F32, tag="uhnh", name=f"uhnh{s}")
        nc.vector.tensor_scalar(uhnh[:], rz[:, 1:2], S(f"hc{s}"),
                                S(f"nh{s}"), OP.mult, OP.add)
        onemu = work.tile([96, 1], F32, tag="onemu", name=f"onemu{s}")
        nc.vector.tensor_scalar(onemu[:], rz[:, 1:2], -1.0, 1.0,
                                OP.mult, OP.add)

        # ---- top eager window (PE: under rz/c ACT) ----
        # conv_s remaining inputs: c-taps, full x-chunks, g_{s-1} tap
        if s > 1:
            for j in range((CD[s - 1] + 127) // 128):
                nc.tensor.matmul(pR[0:128 if j == 0 else 32, s - 1:s],
                                 W(f"cvc{s}_{j}"), W(f"cxh{s}_{j}"),
                                 start=(j == 0), stop=False,
                                 skip_group_check=True)
            for c in range(s - 1):
                cvx_mm(s, c, 128)
            cvx_mm(s, s - 1, 96)  # g_{s-1} tap (after the column clear)
        if s < n_stages:
            emit_gh(s + 1)
            # half the ready full chunks of gi_{s+1} here
            full_cs = list(range(s - 1))
            mid_cs = full_cs[:(len(full_cs) + 1) // 2]
            late_cs = full_cs[len(mid_cs):]
            for c in mid_cs:
                gi_rz(s + 1, c, 128)
                gi_n(s + 1, c, 128, start=not pn_started.get(s + 1, False))
                pn_started[s + 1] = True

        # chain: hn_raw = c*onemu + uhnh ; XHN = clamp(hn_raw) (bf16)
        hnr = work.tile([96, 1], F32, tag="hnr", name=f"hnr{s}")
        nc.vector.tensor_scalar(hnr[:], c_[:], onemu[:], uhnh[:],
                                OP.mult, OP.add)
        nc.vector.tensor_scalar(XHN[:, s - 1:s], hnr[:], -1.0, 1.0,
                                OP.max, OP.min)

        # GLU
        Q = pq.tile([128, 1], F32, tag="q", name=f"Q{s}")
        nc.tensor.matmul(Q[:], W(f"glu{s}"), XHN[:, s - 1:s],
                         start=True, stop=True)
        nc.scalar.activation(SGB[:, s - 1:s], Q[0:96, :], AF.Sigmoid)

        # off-chain f32 clamped hn (scalar operand for the folds below)
        hnf = work.tile([96, 1], F32, tag="hnf", name=f"hnf{s}")
        nc.vector.tensor_scalar(hnf[:], hnr[:], -1.0, 1.0, OP.max, OP.min)
        # conv g-tap: WSC = cw1_g * hn (off-chain), matmul against sg
        WSC = work.tile([96, 128], BF16, tag="wsc", name=f"WSC{s}")
        nc.vector.tensor_scalar(WSC[:], W(f"cvx{s}_{s}"), hnf[:],
                                None, OP.mult)
        if s < n_stages:
            emit_ghn_copy(s + 1)

        nc.tensor.matmul(pR[0:128, s - 1:s], WSC[:], SGB[:, s - 1:s],
                         start=False, stop=True, skip_group_check=(s > 1))

        # ---- mid eager window (PE: fills the cv-tanh wait) ----
        if s < n_stages:
            for c in late_cs:
                gi_rz(s + 1, c, 128)
                gi_n(s + 1, c, 128, start=not pn_started.get(s + 1, False))
                pn_started[s + 1] = True

        # g = hn * sg -> XH g-col (off-chain; feeds gi/cvx/out consumers)
        nc.vector.tensor_scalar(XH[0:96, s:s + 1], SGB[:, s - 1:s],
                                hnf[:], None, OP.mult)

        # ---- late eager window (PE: under cv tanh) ----
        if s < n_stages:
            gi_rz(s + 1, s, 96)
            gi_n(s + 1, s, 96, start=not pn_started.get(s + 1, False))
            pn_started[s + 1] = True
        if n_stages == 5:
            if s == 4:
                for c2 in (0, 1, 2):
                    nc.tensor.matmul(O[:], W(f"out{c2}"), XH[:, c2:c2 + 1],
                                     start=(c2 == 0), stop=False)
            elif s == 5:
                nc.tensor.matmul(O[:], W("out3"), XH[:, 3:4],
                                 start=False, stop=False)
                nc.tensor.matmul(O[:], W("out5"), XH[0:97, 5:6],
                                 start=False, stop=False)

        # conv tanh writes bf16 cv_s directly into XH rows 96:128
        nc.scalar.activation(XH[96:128, s - 1:s], pR[0:32, s - 1:s], AF.Tanh)

        # ---- boundary: last missing gi_{s+1} chunk (g_{s-1}+cv_s) ----
        if s < n_stages:
            gi_rz(s + 1, s - 1, 128, stop=True)
            gi_n(s + 1, s - 1, 128, stop=True)
            # conv_{s+1} chunk s-1 (full) moves to stage s+1 top window

    # ---------------- tail ----------------
    if n_stages == 5:
        nc.tensor.matmul(O[:], W("out4"), XH[:, 4:5], start=False, stop=True)
    else:
        nc.tensor.matmul(O[:], W("out0"), XH[:, 0:1], start=True, stop=True)
    y_sb = work.tile([80, 1], F32, tag="y", name="y_sb")
    nc.vector.tensor_copy(y_sb[:], O[0:80, :])
    nc.sync.dma_start(out=ydram[:], in_=y_sb[:])


_NC_CACHE = None


def _get_nc():
    global _NC_CACHE
    if _NC_CACHE is None:
        _NC_CACHE = _build_nc()
    return _NC_CACHE


def kernel(**inputs) -> np.ndarray:
    from concourse.bass_utils import run_bass_kernel_spmd

    nc = _get_nc()
    in_map = _pack(inputs)
    in_maps = [in_map for _ in range(N_CORES)]
    res = run_bass_kernel_spmd(nc, in_maps, list(range(N_CORES)))
    y = np.asarray(res.results[0]["y"]).reshape(-1)
    return y.reshape(1, 4, 20).astype(np.float32)


# revision 19
# speedup vs baseline: 1.1182x; 1.1182x over previous
"""Trainium2 Bass kernel for nn_CoreDecoderStatefull — v3.

Single-step stateful decoder: dense -> 5x [GRU -> GLU -> concat -> conv1d(k=2)
-> concat] -> out projection.  batch=1; latency-bound serial chain.

v3 changes vs v2 (39.7us):
  * HW-measured cost model (loop-difference microbenchmarks): ACT op ~380ns
    dependent / ~254 independent; DVE tensor_scalar ~170ns but
    scalar_tensor_tensor ~328ns; LDW+MM pair 182ns @96-col stationary but
    70.7ns @128-col (FWL kicks in at NumWeights==128); DMA ~0.7-1.0us fixed
    per transfer, serialized.
  * All 96/80-col weight blocks padded to 128 cols -> FWL halves LDWEIGHTS
    and pairs pipeline 2.6x better.  (conv 32-col blocks stay unpadded.)
  * DMA: 10 transfers -> 5 need-ordered waves (head+stage1 / stile /
    stage2 / stages3-4 / stage5+out), each ~0.3-1.1MB.
  * GRU blend avoids scalar_tensor_tensor: hn = (c*onemu)+uhnh via ONE
    tensor_scalar with two per-partition AP scalars; uhnh=(u*h+nh) and
    onemu=(1-u) precomputed on DVE concurrently with the c tanh.
  * Psum split Prz/Pn per stage: boundary emits r,z cv-row matmuls first
    (stop) so the rz sigmoid starts before the n-gate matmul lands.
  * conv g-tap folded: pre-scale the [96,32] cw1_g block by clamped hn on
    DVE (off-chain, concurrent with GLU matmul) and matmul against sg
    directly -- the g=hn*sg multiply leaves the critical path.
  * x0 tanh writes bf16 XH col directly (no extra DVE move).
"""

import numpy as np
from contextlib import ExitStack

GD = [96, 224, 352, 480, 608]   # GRU input dims per stage
CD = [192, 320, 448, 576, 704]  # conv input dims per stage
N_CORES = 8
N_WAVES = 5


def _bf16(a):
    a = np.ascontiguousarray(np.asarray(a, np.float32))
    u = a.view(np.uint32)
    r = ((u + 0x7FFF + ((u >> 16) & 1)) & 0xFFFF0000).astype(np.uint32)
    return r.view(np.float32)


# ---------------------------------------------------------------------------
# x-vector index mapping: chunk c row r -> index in the reference concat x
# chunk c: rows 0:96 = x0 (c=0) or g_c, rows 96:128 = cv_{c+1}
# ---------------------------------------------------------------------------
def _refidx(c: int, r: int) -> int:
    if r < 96:
        return r if c == 0 else GD[c - 1] + r  # x0 / g_c
    return CD[c] + (r - 96)                    # cv_{c+1}


def _gi_chunks(s):
    # (chunk, rows): full chunks then the g_{s-1}-only last chunk
    return [(c, 128) for c in range(s - 1)] + [(s - 1, 96)]


def _cvx_chunks(s):
    # conv_s x-input chunks; chunk s (rows 0:96, g_s) is the runtime-scaled
    # g-tap; chunks 0..s-2 full, chunk s-1 g-only
    return [(c, 128) for c in range(s - 1)] + [(s - 1, 96), (s, 96)]


_OUT_CHUNKS = [(c, 128) for c in range(5)] + [(5, 97)]

# stage -> wave for that stage's chain weights
_STAGE_WAVE = {1: 0, 2: 1, 3: 2, 4: 3, 5: 4}


# ---------------------------------------------------------------------------
# static layout: 4 bf16 wave slabs + small f32 state tile
# ---------------------------------------------------------------------------
def _layout():
    wt = {}  # name -> (wave, col, rows, ncols)
    wave_cols = [0] * N_WAVES

    def put(name, wave, rows, ncols):
        wt[name] = (wave, wave_cols[wave], rows, ncols)
        wave_cols[wave] += ncols

    # wave 0: everything stage-1-critical; later stages' inputs ride the
    # same wave as that stage's weights
    put("zxh", 0, 81, 1)
    put("dense", 0, 81, 128)
    for s in range(1, 6):
        put(f"hxh{s}", _STAGE_WAVE[s], 97, 1)
    for s in range(1, 6):
        cd = CD[s - 1]
        nch = (cd + 127) // 128
        for j in range(nch):
            rows = min(128, cd - 128 * j)
            if j == nch - 1:
                rows += 1  # aug row = 1.0 (for cvc bias)
            put(f"cxh{s}_{j}", _STAGE_WAVE[s], rows, 1)

    for s in range(1, 6):
        w = _STAGE_WAVE[s]
        for j in range(3):
            put(f"gh{s}_{j}", w, 97, 128)
        for (c, rows) in _gi_chunks(s):
            for j in range(3):
                put(f"gi{s}_{c}_{j}", w, rows, 128)
        put(f"glu{s}", w, 96, 128)
        for (c, rows) in _cvx_chunks(s):
            put(f"cvx{s}_{c}", w, rows, 128 if c == s else 32)
        cd = CD[s - 1]
        nch = (cd + 127) // 128
        for j in range(nch):
            rows = min(128, cd - 128 * j)
            if j == nch - 1:
                rows += 1  # aug bias row
            put(f"cvc{s}_{j}", w, rows, 128 if j == 0 else 32)

    for (c, rows) in _OUT_CHUNKS:
        put(f"out{c}", 4, rows, 128)

    # stile: fp32 state columns (nh, hc, binf per stage)
    st = {}
    scol = 0
    for s in range(1, 6):
        for nm in (f"nh{s}", f"hc{s}", f"binf{s}"):
            st[nm] = scol
            scol += 1
    return wt, wave_cols, st, scol


_WT, _WAVE_COLS, _ST, _ST_COLS = _layout()


# ---------------------------------------------------------------------------
# host-side packing
# ---------------------------------------------------------------------------
def _noise_vectors():
    # NOTE: must follow the exact same jax.random path as the reference.
    import jax
    import jax.numpy as jnp

    vs = {}
    for i in range(16):
        n = 96 if (i == 0 or i % 3 != 0) else 32
        u = jax.random.uniform(
            jax.random.fold_in(jax.random.key(42), i), (1, n),
            dtype=jnp.float32
        )
        vs[i] = (np.asarray(u).reshape(-1) - 0.5) / np.float32(127.0)
    return vs


def _to_ml_bf16(a):
    import ml_dtypes

    return np.asarray(a, np.float32).astype(ml_dtypes.bfloat16)


def _pack(inp):
    f32 = np.float32
    nv = _noise_vectors()
    # nu_x: constant noise folded into the device x (x0, g_s, cv_s sites)
    nux = np.zeros(736, f32)
    nux[0:96] = nv[0]
    for s in range(1, 6):
        nux[GD[s - 1]:GD[s - 1] + 96] = nv[3 * s - 1]   # g_s
        nux[CD[s - 1]:CD[s - 1] + 32] = nv[3 * s]       # cv_s
    waves = [np.zeros((128, c), f32) for c in _WAVE_COLS]
    stile = np.zeros((128, _ST_COLS), f32)

    def wfill(name, block):
        wave, col, rows, ncols = _WT[name]
        assert block.shape[0] == rows, (name, block.shape, rows)
        assert block.shape[1] <= ncols, (name, block.shape, ncols)
        waves[wave][:rows, col:col + block.shape[1]] = _bf16(block)

    def sfill(name, vec):
        stile[:vec.shape[0], _ST[name]] = vec

    # dense: rows 0:80 = w.T, row 80 = bias
    blk = np.zeros((81, 96), f32)
    blk[:80] = inp["w_dense"].T
    blk[80] = inp["b_dense"]
    wfill("dense", blk)

    for s in range(1, 6):
        wi = inp[f"g{s}_wi"].astype(f32)
        wh = inp[f"g{s}_wh"].astype(f32)
        bi = inp[f"g{s}_bi"].astype(f32)
        bh = inp[f"g{s}_bh"].astype(f32)
        nux_s = nux[:GD[s - 1]]
        fold = wi @ nux_s  # (288,)
        # gh blocks: rows 0:96 wh.T, row 96 bias(+fold for r,z)
        for j in range(3):
            blk = np.zeros((97, 96), f32)
            blk[:96] = wh[96 * j:96 * (j + 1), :].T
            if j < 2:
                blk[96] = (bi + bh)[96 * j:96 * (j + 1)] + fold[96 * j:96 * (j + 1)]
            else:
                blk[96] = bh[192:288]
            wfill(f"gh{s}_{j}", blk)
        sfill(f"binf{s}", bi[192:288] + fold[192:288])
        sfill(f"nh{s}", nv[3 * s - 2])
        sfill(f"hc{s}", inp[f"h{s}"].reshape(-1).astype(f32))

        # gi chunks (row-permuted)
        for (c, rows) in _gi_chunks(s):
            ridx = [_refidx(c, r) for r in range(rows)]
            for j in range(3):
                wfill(f"gi{s}_{c}_{j}", wi[96 * j:96 * (j + 1), ridx].T)

        wfill(f"glu{s}", inp[f"glu{s}_w"].T.astype(f32))

        cw = inp[f"cv{s}_w"].astype(f32)
        cw0, cw1 = cw[:, :, 0], cw[:, :, 1]
        cd = CD[s - 1]
        # conv x-taps (chunk s = the g_s tap, runtime-scaled on device)
        for (c, rows) in _cvx_chunks(s):
            ridx = [_refidx(c, r) for r in range(rows)]
            wfill(f"cvx{s}_{c}", cw1[:, ridx].T)
        # conv c-taps; last chunk aug row = cb + cw1 @ nux fold
        nch = (cd + 127) // 128
        for j in range(nch):
            rows = min(128, cd - 128 * j)
            blk_w = cw0[:, 128 * j:128 * j + rows].T
            if j == nch - 1:
                blk = np.zeros((rows + 1, 32), f32)
                blk[:rows] = blk_w
                blk[rows] = inp[f"cv{s}_b"].astype(f32) + cw1 @ nux[:cd]
                wfill(f"cvc{s}_{j}", blk)
            else:
                wfill(f"cvc{s}_{j}", blk_w)

    w_out = inp["w_out"].astype(f32)
    for (c, rows) in _OUT_CHUNKS:
        if c < 5:
            ridx = [_refidx(c, r) for r in range(rows)]
            wfill(f"out{c}", w_out[:, ridx].T)
        else:
            blk = np.zeros((97, 80), f32)
            ridx = [GD[4] + r for r in range(96)]  # g5 dims
            blk[:96] = w_out[:, ridx].T
            blk[96] = inp["b_out"].astype(f32) + w_out @ nux
            wfill(f"out{c}", blk)

    # bf16 input columns (in wave 0)
    zv = np.zeros((81, 1), f32)
    zv[:80, 0] = inp["z"].reshape(-1)
    zv[80, 0] = 1.0
    wfill("zxh", zv)
    for s in range(1, 6):
        hv = np.zeros((97, 1), f32)
        hv[:96, 0] = inp[f"h{s}"].reshape(-1)
        hv[96, 0] = 1.0
        wfill(f"hxh{s}", hv)
        cv_in = inp[f"c{s}"].reshape(-1).astype(f32)
        cd = CD[s - 1]
        nch = (cd + 127) // 128
        for j in range(nch):
            rows = min(128, cd - 128 * j)
            if j == nch - 1:
                v = np.zeros((rows + 1, 1), f32)
                v[:rows, 0] = cv_in[128 * j:128 * j + rows]
                v[rows, 0] = 1.0
                wfill(f"cxh{s}_{j}", v)
            else:
                wfill(f"cxh{s}_{j}", cv_in[128 * j:128 * j + rows].reshape(-1, 1))

    m = {f"wave{i}": _to_ml_bf16(waves[i]) for i in range(N_WAVES)}
    m["stile"] = stile
    return m


# ---------------------------------------------------------------------------
# device program
# ---------------------------------------------------------------------------
def _build_nc(loop_iters=None, dma_only=False, compute_only=False,
              n_stages=5):
    from concourse import bacc, tile, mybir

    F32 = mybir.dt.float32
    BF16 = mybir.dt.bfloat16
    AF = mybir.ActivationFunctionType
    OP = mybir.AluOpType

    nc = bacc.Bacc("TRN2", target_bir_lowering=False, debug=False,
                   num_devices=N_CORES)
    wdram = {i: nc.dram_tensor(f"wave{i}", [128, _WAVE_COLS[i]], BF16,
                               kind="ExternalInput") for i in range(N_WAVES)}
    sdram = nc.dram_tensor("stile", [128, _ST_COLS], F32, kind="ExternalInput")
    ydram = nc.dram_tensor("y", [80, 1], F32, kind="ExternalOutput")

    with tile.TileContext(nc) as tc, ExitStack() as ctx:
        dbuf = 2 if loop_iters is not None else 1
        wpool = ctx.enter_context(tc.tile_pool(name="wpool", bufs=dbuf))
        spool = ctx.enter_context(tc.tile_pool(name="spool", bufs=dbuf))
        work = ctx.enter_context(tc.tile_pool(name="work", bufs=2))
        xpool = ctx.enter_context(tc.tile_pool(name="xpool", bufs=1))
        prz = ctx.enter_context(tc.tile_pool(name="prz", bufs=2, space="PSUM"))
        ppn = ctx.enter_context(tc.tile_pool(name="ppn", bufs=2, space="PSUM"))
        pgh = ctx.enter_context(tc.tile_pool(name="pgh", bufs=1, space="PSUM"))
        pcv = ctx.enter_context(tc.tile_pool(name="pcv", bufs=1, space="PSUM"))
        pq = ctx.enter_context(tc.tile_pool(name="pq", bufs=1, space="PSUM"))
        pout = ctx.enter_context(tc.tile_pool(name="pout", bufs=1, space="PSUM"))

        if loop_iters is not None:
            ctx.enter_context(tc.For_i(0, loop_iters, 1))

        # single warm sigmoid: pins the table set to sigmoid_and_others
        # (includes tanh) so exactly one ACT_TABLE_LOAD is emitted
        warm = work.tile([1, 1], F32, tag="warm", name="warm")
        nc.vector.memset(warm[:], 0.0)
        nc.scalar.activation(warm[:], warm[:], AF.Sigmoid)

        XH = xpool.tile([128, 6], BF16, tag="XH", name="XH")
        XHN = xpool.tile([96, 5], BF16, tag="XHN", name="XHN")
        SGB = xpool.tile([96, 5], BF16, tag="SGB", name="SGB")
        nc.vector.memset(XH[96:97, 5:6], 1.0)  # aug row for out bias

        stile = spool.tile([128, _ST_COLS], F32, tag="stile", name="stile")
        wt = {}
        for i in range(N_WAVES):
            wt[i] = wpool.tile([128, _WAVE_COLS[i]], BF16, tag=f"w{i}",
                               name=f"wt{i}")
        if not compute_only:
            nc.sync.dma_start(out=wt[0][:], in_=wdram[0][:])
            nc.sync.dma_start(out=stile[:], in_=sdram[:])
            for i in range(1, N_WAVES):
                nc.sync.dma_start(out=wt[i][:], in_=wdram[i][:])
        else:
            nc.vector.memset(stile[:, 0:1], 0.01)
            for i in range(N_WAVES):
                nc.vector.memset(wt[i][:, 0:1], 0.01)

        def W(name):
            wave, col, rows, ncols = _WT[name]
            return wt[wave][0:rows, col:col + ncols]

        def S(name):
            return stile[0:96, _ST[name]:_ST[name] + 1]

        if not dma_only:
            _emit_compute(nc, tc, work, xpool, prz, ppn, pgh, pcv, pq, pout,
                          W, S, XH, XHN, SGB, n_stages, mybir, ydram)

    nc.compile()
    return nc


def _emit_compute(nc, tc, work, xpool, prz, ppn, pgh, pcv, pq, pout,
                  W, S, XH, XHN, SGB, n_stages, mybir, ydram):
    F32 = mybir.dt.float32
    BF16 = mybir.dt.bfloat16
    AF = mybir.ActivationFunctionType
    OP = mybir.AluOpType

    Prz = {}
    Pn = {}
    ghn = {}
    phn = {}

    def emit_ghn_copy(t):
        ghn[t] = work.tile([96, 1], F32, tag="ghn", name=f"ghn{t}")
        nc.vector.tensor_copy(ghn[t][:], phn[t][0:96, :])

    def emit_gh(t):
        """gh r,z into Prz[t] (opens the group), gh n into its own psum tile
        (closed immediately; DVE copies it out right after)."""
        Prz[t] = prz.tile([128, 2], F32, tag="prz", name=f"Prz{t}")
        for j in range(2):
            nc.tensor.matmul(Prz[t][:, j:j + 1], W(f"gh{t}_{j}"),
                             W(f"hxh{t}"), start=(j == 0), stop=False)
        phn[t] = pgh.tile([128, 1], F32, tag="gh", name=f"Pghn{t}")
        nc.tensor.matmul(phn[t][:], W(f"gh{t}_2"), W(f"hxh{t}"),
                         start=True, stop=True)

    def gi_rz(t, c, rows, stop=False):
        for j in range(2):
            nc.tensor.matmul(Prz[t][:, j:j + 1], W(f"gi{t}_{c}_{j}"),
                             XH[0:rows, c:c + 1],
                             start=False, stop=(stop and j == 1))

    def gi_n(t, c, rows, start=False, stop=False):
        if start:
            Pn[t] = ppn.tile([128, 1], F32, tag="pn", name=f"Pn{t}")
        nc.tensor.matmul(Pn[t][:], W(f"gi{t}_{c}_2"), XH[0:rows, c:c + 1],
                         start=start, stop=stop)

    def cvx_mm(s, c, rows, stop=False):
        nc.tensor.matmul(pR[0:32, s - 1:s], W(f"cvx{s}_{c}"),
                         XH[0:rows, c:c + 1], start=False, stop=stop,
                         skip_group_check=(s > 1))

    # ---------------- head ----------------
    pd = pq.tile([128, 1], F32, tag="q", name="pdense")
    nc.tensor.matmul(pd[:], W("dense"), W("zxh"), start=True, stop=True)
    emit_gh(1)
    emit_ghn_copy(1)
    nc.scalar.activation(XH[0:96, 0:1], pd[0:96, :], AF.Tanh)  # x0 (bf16)

    gi_rz(1, 0, 96, stop=True)
    gi_n(1, 0, 96, start=True, stop=True)

    pR = pcv.tile([128, 5], F32, tag="cv", name="pR")
    for j in range((CD[0] + 127) // 128):
        nc.tensor.matmul(pR[0:128 if j == 0 else 32, 0:1], W(f"cvc1_{j}"),
                         W(f"cxh1_{j}"), start=(j == 0), stop=False)
    cvx_mm(1, 0, 96)

    O = pout.tile([128, 1], F32, tag="out", name="Oout")

    # ---------------- stage chain ----------------
    pn_started = {1: True}
    for s in range(1, n_stages + 1):
        # ---- DVE pre-chain (concurrent with rz/c ACT) ----
        ginb = work.tile([96, 1], F32, tag="ginb", name=f"ginb{s}")
        nc.vector.tensor_scalar(ginb[:], Pn[s][0:96, :], S(f"binf{s}"),
                                None, OP.add)
        rz = work.tile([96, 2], F32, tag="rz", name=f"rz{s}")
        nc.scalar.activation(rz[:], Prz[s][0:96, :], AF.Sigmoid)
        c_ = work.tile([96, 1], F32, tag="c_", name=f"c{s}_")
        nc.scalar.activation(c_[:], rz[:, 0:1], AF.Tanh,
                             bias=ginb[:], scale=ghn[s][:])
        # during c tanh: uhnh = u*h + nh ; onemu = 1 - u
        uhnh = work.tile([96, 1], F32, tag="uhnh", name=f"uhnh{s}")
        nc.vector.tensor_scalar(uhnh[:], rz[:, 1:2], S(f"hc{s}"),
                                S(f"nh{s}"), OP.mult, OP.add)
        onemu = work.tile([96, 1], F32, tag="onemu", name=f"onemu{s}")
        nc.vector.tensor_scalar(onemu[:], rz[:, 1:2], -1.0, 1.0,
                                OP.mult, OP.add)

        # ---- top eager window (PE: under rz/c ACT) ----
        # conv_s remaining inputs: c-taps, full x-chunks, g_{s-1} tap
        if s > 1:
            for j in range((CD[s - 1] + 127) // 128):
                nc.tensor.matmul(pR[0:128 if j == 0 else 32, s - 1:s],
                                 W(f"cvc{s}_{j}"), W(f"cxh{s}_{j}"),
                                 start=(j == 0), stop=False,
                                 skip_group_check=True)
            for c in range(s - 1):
                cvx_mm(s, c, 128)
            cvx_mm(s, s - 1, 96)  # g_{s-1} tap (after the column clear)
        if s < n_stages:
            emit_gh(s + 1)
            # half the ready full chunks of gi_{s+1} here
            full_cs = list(range(s - 1))
            mid_cs = full_cs[:(len(full_cs) + 1) // 2]
            late_cs = full_cs[len(mid_cs):]
            for c in mid_cs:
                gi_rz(s + 1, c, 128)
                gi_n(s + 1, c, 128, start=not pn_started.get(s + 1, False))
                pn_started[s + 1] = True

        # chain: hn_raw = c*onemu + uhnh ; XHN = clamp(hn_raw) (bf16)
        hnr = work.tile([96, 1], F32, tag="hnr", name=f"hnr{s}")
        nc.vector.tensor_scalar(hnr[:], c_[:], onemu[:], uhnh[:],
                                OP.mult, OP.add)
        nc.vector.tensor_scalar(XHN[:, s - 1:s], hnr[:], -1.0, 1.0,
                                OP.max, OP.min)

        # GLU
        Q = pq.tile([128, 1], F32, tag="q", name=f"Q{s}")
        nc.tensor.matmul(Q[:], W(f"glu{s}"), XHN[:, s - 1:s],
                         start=True, stop=True)
        nc.scalar.activation(SGB[:, s - 1:s], Q[0:96, :], AF.Sigmoid)

        # off-chain f32 clamped hn (scalar operand for the folds below)
        hnf = work.tile([96, 1], F32, tag="hnf", name=f"hnf{s}")
        nc.vector.tensor_scalar(hnf[:], hnr[:], -1.0, 1.0, OP.max, OP.min)
        # conv g-tap: WSC = cw1_g * hn (off-chain), matmul against sg
        WSC = work.tile([96, 128], BF16, tag="wsc", name=f"WSC{s}")
        nc.vector.tensor_scalar(WSC[:], W(f"cvx{s}_{s}"), hnf[:],
                                None, OP.mult)
        if s < n_stages:
            emit_ghn_copy(s + 1)

        nc.tensor.matmul(pR[0:128, s - 1:s], WSC[:], SGB[:, s - 1:s],
                         start=False, stop=True, skip_group_check=(s > 1))

        # ---- mid eager window (PE: fills the cv-tanh wait) ----
        if s < n_stages:
            for c in late_cs:
                gi_rz(s + 1, c, 128)
                gi_n(s + 1, c, 128, start=not pn_started.get(s + 1, False))
                pn_started[s + 1] = True

        # g = hn * sg -> XH g-col (off-chain; feeds gi/cvx/out consumers)
        nc.vector.tensor_scalar(XH[0:96, s:s + 1], SGB[:, s - 1:s],
                                hnf[:], None, OP.mult)

        # ---- late eager window (PE: under cv tanh) ----
        if s < n_stages:
            gi_rz(s + 1, s, 96)
            gi_n(s + 1, s, 96, start=not pn_started.get(s + 1, False))
            pn_started[s + 1] = True
        if n_stages == 5:
            if s == 4:
                for c2 in (0, 1, 2):
                    nc.tensor.matmul(O[:], W(f"out{c2}"), XH[:, c2:c2 + 1],
                                     start=(c2 == 0), stop=False)
            elif s == 5:
                nc.tensor.matmul(O[:], W("out3"), XH[:, 3:4],
                                 start=False, stop=False)
                nc.tensor.matmul(O[:], W("out5"), XH[0:97, 5:6],
                                 start=False, stop=False)

        # conv tanh writes bf16 cv_s directly into XH rows 96:128
        nc.scalar.activation(XH[96:128, s - 1:s], pR[0:32, s - 1:s], AF.Tanh)

        # ---- boundary: last missing gi_{s+1} chunk (g_{s-1}+cv_s) ----
        if s < n_stages:
            gi_rz(s + 1, s - 1, 128, stop=True)
            gi_n(s + 1, s - 1, 128, stop=True)
            # conv_{s+1} chunk s-1 (full) moves to stage s+1 top window

    # ---------------- tail ----------------
    if n_stages == 5:
        nc.tensor.matmul(O[:], W("out4"), XH[:, 4:5], start=False, stop=True)
    else:
        nc.tensor.matmul(O[:], W("out0"), XH[:, 0:1], start=True, stop=True)
    y_sb = work.tile([80, 1], F32, tag="y", name="y_sb")
    nc.vector.tensor_copy(y_sb[:], O[0:80, :])
    nc.sync.dma_start(out=ydram[:], in_=y_sb[:])


_NC_CACHE = None


def _get_nc():
    global _NC_CACHE
    if _NC_CACHE is None:
        _NC_CACHE = _build_nc()
    return _NC_CACHE


def kernel(**inputs) -> np.ndarray:
    from concourse.bass_utils import run_bass_kernel_spmd

    nc = _get_nc()
    in_map = _pack(inputs)
    in_maps = [in_map for _ in range(N_CORES)]
    res = run_bass_kernel_spmd(nc, in_maps, list(range(N_CORES)))
    y = np.asarray(res.results[0]["y"]).reshape(-1)
    return y.reshape(1, 4, 20).astype(np.float32)
